# revision 1
# baseline (speedup 1.0000x reference)
"""DecodeDetections kernel for Trainium2 (Bass/Tile), 8-core data-parallel.

Full input y_pred [64, 8732, 33] f32 -> output [64, 200, 6] f32.
Each of the 8 NeuronCores handles 8 batch items ("tokens"):
  decode SSD boxes, per-class scores, exact top-200 (jax top_k tie rules).

Pipeline per core (tokens t=0..7 on partition groups [16t, 16t+16)):
  1. DMA raw rows into SBUF [128, 546*33] (box-blocked per partition).
  2. DVE strided copy -> scores S [128, 10920]  (v = i*10920 + j*20 + cls).
  3. 3x gpsimd topk (vocab 58240) -> per-chunk top-256 (values+indices).
  4. gpsimd topk on the 768 chunk-winners (padded vocab 57344) -> exact
     per-token top-256 values, sorted ascending.
  5. Stage chunk-topk indices to DRAM; per-winner indirect gather of v.
  6. v -> (cls, n); indirect gather of the winners' 33-ch rows; decode boxes.
  7. Exact rank (value desc, m=cls*8732+n asc; +-2 tie window) and
     indirect scatter of rows [class_id, conf, xmin, ymin, xmax, ymax]
     to out[t*200 + rank] with bounds_check dropping rank >= 200.
"""

import os
import sys

for _p in ("/opt/trn_rl_repo", "/root/.axon_site/_ro/trn_rl_repo"):
    if os.path.isdir(_p) and _p not in sys.path:
        sys.path.insert(0, _p)

import numpy as np

import concourse.bass as bass
import concourse.bacc as bacc
import concourse.bass_isa as bass_isa
import concourse.mybir as mybir
import concourse.tile as tile
from concourse.bass_utils import run_bass_kernel_spmd

# problem constants
B = 64
NBOX = 8732
NCH = 33
NCLS = 20          # foreground classes (channels 1..20)
TOPK = 200
NCORES = 8
TPC = 8            # tokens (batch items) per core

NB = 546           # boxes per partition (546*16 = 8736 >= 8732)
NBP = 8736         # padded boxes per token in DRAM
RAWC = NB * NCH    # 18018
SCOLS = NB * NCLS  # 10920 score cols per partition
CHUNK = 58240      # stage-1 topk vocab (SCOLS/3 * 16)
CCOLS = CHUNK // 16  # 3640
V2 = 50176         # stage-2 topk vocab (minimum legal > 50000, %128 == 0)
V2C = V2 // 16     # 3136
IMG = 512.0


def _topk(nc, out_ap, in_ap, tokens, vocab, k=256):
    _in = nc.gpsimd.lower_ap(in_ap, for_isa=True)
    _out = nc.gpsimd.lower_ap(out_ap, for_isa=True)
    return nc.gpsimd.add_instruction(
        bass_isa.InstTopk(name=f"I-{nc.next_id()}", ins=[_in], outs=[_out],
                          _tokens=tokens, _n=vocab, _k=k))


class _Helper:
    """Float-exact integer div/mod on [128, W] f32 tiles."""

    def __init__(self, nc, pool, w):
        self.nc, self.pool, self.w = nc, pool, w
        self.t1 = pool.tile([128, w], mybir.dt.float32, name="hlp_t1")
        self.ti = pool.tile([128, w], mybir.dt.int32, name="hlp_ti")
        self.t2 = pool.tile([128, w], mybir.dt.float32, name="hlp_t2")

    def fdiv(self, out, in_, d):
        """out = floor(in_/d) for integer-valued f32 in_ >= 0 (exact)."""
        nc = self.nc
        nc.vector.tensor_scalar(self.t1[:], in_, float((1 + 2.0 ** -20) / d),
                                scalar2=None, op0=mybir.AluOpType.mult)
        nc.vector.tensor_copy(self.ti[:], self.t1[:])   # f32 -> i32
        nc.vector.tensor_copy(out, self.ti[:])          # i32 -> f32
        nc.vector.tensor_scalar(self.t1[:], out, float(d),
                                scalar2=None, op0=mybir.AluOpType.mult)
        nc.vector.tensor_tensor(self.t2[:], self.t1[:], in_,
                                op=mybir.AluOpType.is_gt)
        nc.vector.tensor_tensor(out, out, self.t2[:],
                                op=mybir.AluOpType.subtract)

    def fmod(self, out, in_, quot, d):
        """out = in_ - quot*d (exact)."""
        nc = self.nc
        nc.vector.tensor_scalar(self.t1[:], quot, float(d),
                                scalar2=None, op0=mybir.AluOpType.mult)
        nc.vector.tensor_tensor(out, in_, self.t1[:],
                                op=mybir.AluOpType.subtract)


def build_kernel():
    nc = bacc.Bacc("TRN2", target_bir_lowering=False, debug=False)
    y = nc.dram_tensor("y", [TPC * NBP, NCH], mybir.dt.float32,
                       kind="ExternalInput")
    out = nc.dram_tensor("out", [TPC * TOPK, 6], mybir.dt.float32,
                         kind="ExternalOutput")

    with tile.TileContext(nc) as tc:
        with tc.tile_pool(name="sbuf", bufs=1) as pool, \
             tc.tile_pool(name="dram", bufs=1, space="DRAM") as dpool:

            raw = pool.tile([128, RAWC], mybir.dt.float32)
            S = pool.tile([128, SCOLS], mybir.dt.float32)
            tk1 = pool.tile([128, 96], mybir.dt.uint32)
            pad2 = pool.tile([128, V2C], mybir.dt.float32)
            tk2 = pool.tile([128, 32], mybir.dt.uint32)

            yv = y[:].rearrange("(t i b) c -> t i (b c)", t=TPC, i=16)

            nc.gpsimd.memset(pad2[:], 0.0)

            # 3 box-range chunks: DMA -> score copy -> chunk topk
            NBC = NB // 3  # 182 boxes per chunk
            for j in range(3):
                with nc.named_scope(f"load{j}"):
                    nc.sync.dma_start(
                        raw[:, j * NBC * NCH:(j + 1) * NBC * NCH],
                        yv[:, :, j * NBC * NCH:(j + 1) * NBC * NCH])
                with nc.named_scope(f"scopy{j}"):
                    nc.vector.tensor_copy(
                        S[:, j * CCOLS:(j + 1) * CCOLS].rearrange(
                            "p (b c) -> p b c", c=NCLS),
                        raw[:].rearrange("p (b c) -> p b c", c=NCH)[
                            :, j * NBC:(j + 1) * NBC, 1:1 + NCLS])
                with nc.named_scope(f"topk{j}"):
                    _topk(nc, tk1[:, j * 32:(j + 1) * 32],
                          S[:, j * CCOLS:(j + 1) * CCOLS],
                          tokens=TPC, vocab=CHUNK)

            # stage-2: exact top-256 of the 768 chunk winners per token
            nc.vector.tensor_copy(
                pad2[:, 0:48].rearrange("p (j c) -> p j c", j=3),
                tk1[:].bitcast(mybir.dt.float32).rearrange(
                    "p (j b) -> p j b", j=3)[:, :, 0:16])
            with nc.named_scope("topk4"):
                _topk(nc, tk2[:], pad2[:], tokens=TPC, vocab=V2)

            # stage chunk-topk indices to DRAM for the v-lookup
            tk1d = dpool.tile([128 * 96, 1], mybir.dt.uint32)
            nc.sync.dma_start(
                tk1d[:].rearrange("(p c) o -> p (c o)", p=128), tk1[:])

            # per-partition constants: t = p // 16
            pidx = pool.tile([128, 1], mybir.dt.int32)
            nc.gpsimd.iota(pidx[:], pattern=[[0, 1]], base=0,
                           channel_multiplier=1)
            pf = pool.tile([128, 1], mybir.dt.float32)
            nc.vector.tensor_copy(pf[:], pidx[:])
            h1 = _Helper(nc, pool, 1)
            tf = pool.tile([128, 1], mybir.dt.float32)
            h1.fdiv(tf[:], pf[:], 16)
            t1536 = pool.tile([128, 1], mybir.dt.float32)
            nc.vector.tensor_scalar(t1536[:], tf[:], 96.0 * 16, scalar2=None,
                                    op0=mybir.AluOpType.mult)
            t8736 = pool.tile([128, 1], mybir.dt.float32)
            nc.vector.tensor_scalar(t8736[:], tf[:], float(NBP), scalar2=None,
                                    op0=mybir.AluOpType.mult)

            # winners: q2 + value (full-width tiles, processed in two
            # independent 8-slot halves so Pool gathers overlap DVE math)
            q2f = pool.tile([128, 16], mybir.dt.float32)
            conf = pool.tile([128, 16], mybir.dt.float32)
            cls = pool.tile([128, 16], mybir.dt.float32)
            m_ = pool.tile([128, 16], mybir.dt.float32)
            vidx = pool.tile([128, 16], mybir.dt.uint32)
            Fu = pool.tile([128, 16], mybir.dt.uint32)
            rowu = pool.tile([128, 16], mybir.dt.uint32)
            enc = pool.tile([128, 16, NCH], mybir.dt.float32)
            rows6 = pool.tile([128, 16, 6], mybir.dt.float32)

            import math as _math
            EXP_C = [1.0 / _math.factorial(kk) for kk in range(11)]

            for hid, hs in enumerate((slice(0, 8), slice(8, 16))):
                hh = _Helper(nc, pool, 8)
                nc.vector.tensor_copy(q2f[:, hs], tk2[:, 16 + hs.start:16 + hs.stop])
                nc.vector.tensor_copy(
                    conf[:, hs], tk2[:, hs].bitcast(mybir.dt.float32))
                i2 = pool.tile([128, 8], mybir.dt.float32, name=f"i2_{hid}")
                c2 = pool.tile([128, 8], mybir.dt.float32, name=f"c2_{hid}")
                jjh = pool.tile([128, 8], mybir.dt.float32, name=f"jjh_{hid}")
                cch = pool.tile([128, 8], mybir.dt.float32, name=f"cch_{hid}")
                hh.fdiv(i2[:], q2f[:, hs], V2C)
                hh.fmod(c2[:], q2f[:, hs], i2[:], V2C)
                hh.fdiv(jjh[:], c2[:], 16)
                hh.fmod(cch[:], c2[:], jjh[:], 16)

                # F = (16t + i2)*96 + 32j + 16 + c
                Fh = pool.tile([128, 8], mybir.dt.float32, name=f"Fh_{hid}")
                tmph = pool.tile([128, 8], mybir.dt.float32, name=f"tmph_{hid}")
                nc.vector.tensor_scalar(Fh[:], i2[:], 96.0, scalar2=None,
                                        op0=mybir.AluOpType.mult)
                nc.vector.tensor_scalar(tmph[:], jjh[:], 32.0, scalar2=16.0,
                                        op0=mybir.AluOpType.mult,
                                        op1=mybir.AluOpType.add)
                nc.vector.tensor_tensor(Fh[:], Fh[:], tmph[:],
                                        op=mybir.AluOpType.add)
                nc.vector.tensor_tensor(Fh[:], Fh[:], cch[:],
                                        op=mybir.AluOpType.add)
                nc.vector.tensor_scalar(Fh[:], Fh[:], t1536[:, 0:1],
                                        scalar2=None, op0=mybir.AluOpType.add)
                nc.vector.tensor_copy(Fu[:, hs], Fh[:])

                with nc.named_scope(f"gather_v{hid}"):
                    for k in range(hs.start, hs.stop):
                        nc.gpsimd.indirect_dma_start(
                            out=vidx[:, k:k + 1], out_offset=None, in_=tk1d[:],
                            in_offset=bass.IndirectOffsetOnAxis(
                                ap=Fu[:, k:k + 1], axis=0),
                            bounds_check=128 * 96 - 1, oob_is_err=False)

                vfh = pool.tile([128, 8], mybir.dt.float32, name=f"vfh_{hid}")
                ivh = pool.tile([128, 8], mybir.dt.float32, name=f"ivh_{hid}")
                rvh = pool.tile([128, 8], mybir.dt.float32, name=f"rvh_{hid}")
                nc.vector.tensor_copy(vfh[:], vidx[:, hs])
                hh.fdiv(ivh[:], vfh[:], CCOLS)
                hh.fmod(rvh[:], vfh[:], ivh[:], CCOLS)
                scolh = pool.tile([128, 8], mybir.dt.float32, name=f"scolh_{hid}")
                nc.vector.tensor_scalar(tmph[:], jjh[:], float(CCOLS),
                                        scalar2=None, op0=mybir.AluOpType.mult)
                nc.vector.tensor_tensor(scolh[:], rvh[:], tmph[:],
                                        op=mybir.AluOpType.add)
                jbh = pool.tile([128, 8], mybir.dt.float32, name=f"jbh_{hid}")
                hh.fdiv(jbh[:], scolh[:], NCLS)
                hh.fmod(cls[:, hs], scolh[:], jbh[:], NCLS)
                nh = pool.tile([128, 8], mybir.dt.float32, name=f"nh_{hid}")
                nc.vector.tensor_scalar(nh[:], ivh[:], float(NB), scalar2=None,
                                        op0=mybir.AluOpType.mult)
                nc.vector.tensor_tensor(nh[:], nh[:], jbh[:],
                                        op=mybir.AluOpType.add)
                nc.vector.tensor_scalar(m_[:, hs], cls[:, hs], float(NBOX),
                                        scalar2=None, op0=mybir.AluOpType.mult)
                nc.vector.tensor_tensor(m_[:, hs], m_[:, hs], nh[:],
                                        op=mybir.AluOpType.add)
                nc.vector.tensor_scalar(nh[:], nh[:], t8736[:, 0:1],
                                        scalar2=None, op0=mybir.AluOpType.add)
                nc.vector.tensor_copy(rowu[:, hs], nh[:])

                with nc.named_scope(f"gather_rows{hid}"):
                    for k in range(hs.start, hs.stop):
                        nc.gpsimd.indirect_dma_start(
                            out=enc[:, k, :], out_offset=None, in_=y[:],
                            in_offset=bass.IndirectOffsetOnAxis(
                                ap=rowu[:, k:k + 1], axis=0),
                            bounds_check=TPC * NBP - 1, oob_is_err=False)

                # decode boxes for this half (enc ch 21..32)
                def ch(k):
                    return enc[:, hs, 21 + k]

                cx = pool.tile([128, 8], mybir.dt.float32, name=f"cx_{hid}")
                cy = pool.tile([128, 8], mybir.dt.float32, name=f"cy_{hid}")
                we = pool.tile([128, 8], mybir.dt.float32, name=f"we_{hid}")
                he = pool.tile([128, 8], mybir.dt.float32, name=f"he_{hid}")
                nc.vector.tensor_tensor(cx[:], ch(0), ch(8),
                                        op=mybir.AluOpType.mult)
                nc.vector.tensor_tensor(cx[:], cx[:], ch(6),
                                        op=mybir.AluOpType.mult)
                nc.vector.tensor_tensor(cx[:], cx[:], ch(4),
                                        op=mybir.AluOpType.add)
                nc.vector.tensor_tensor(cy[:], ch(1), ch(9),
                                        op=mybir.AluOpType.mult)
                nc.vector.tensor_tensor(cy[:], cy[:], ch(7),
                                        op=mybir.AluOpType.mult)
                nc.vector.tensor_tensor(cy[:], cy[:], ch(5),
                                        op=mybir.AluOpType.add)
                nc.vector.tensor_tensor(we[:], ch(2), ch(10),
                                        op=mybir.AluOpType.mult)
                nc.vector.tensor_tensor(he[:], ch(3), ch(11),
                                        op=mybir.AluOpType.mult)
                # exp via degree-10 Taylor Horner (~1 ulp)
                xe = pool.tile([128, 16], mybir.dt.float32, name=f"xe_{hid}")
                nc.vector.tensor_copy(xe[:, 0:8], we[:])
                nc.vector.tensor_copy(xe[:, 8:16], he[:])
                acc = pool.tile([128, 16], mybir.dt.float32, name=f"acc_{hid}")
                nc.vector.memset(acc[:], EXP_C[10])
                for kk in range(9, -1, -1):
                    nc.vector.tensor_tensor(acc[:], acc[:], xe[:],
                                            op=mybir.AluOpType.mult)
                    nc.vector.tensor_scalar(acc[:], acc[:], EXP_C[kk],
                                            scalar2=None,
                                            op0=mybir.AluOpType.add)
                nc.vector.tensor_tensor(we[:], acc[:, 0:8], ch(6),
                                        op=mybir.AluOpType.mult)
                nc.vector.tensor_tensor(he[:], acc[:, 8:16], ch(7),
                                        op=mybir.AluOpType.mult)

                nc.vector.tensor_scalar(rows6[:, hs, 0], cls[:, hs], 1.0,
                                        scalar2=None, op0=mybir.AluOpType.add)
                nc.vector.tensor_copy(rows6[:, hs, 1], conf[:, hs])
                cxs = pool.tile([128, 8], mybir.dt.float32, name=f"cxs_{hid}")
                whs = pool.tile([128, 8], mybir.dt.float32, name=f"whs_{hid}")
                nc.vector.tensor_scalar(cxs[:], cx[:], IMG, scalar2=None,
                                        op0=mybir.AluOpType.mult)
                nc.vector.tensor_scalar(whs[:], we[:], IMG / 2, scalar2=None,
                                        op0=mybir.AluOpType.mult)
                nc.vector.tensor_tensor(rows6[:, hs, 2], cxs[:], whs[:],
                                        op=mybir.AluOpType.subtract)
                nc.vector.tensor_tensor(rows6[:, hs, 4], cxs[:], whs[:],
                                        op=mybir.AluOpType.add)
                nc.vector.tensor_scalar(cxs[:], cy[:], IMG, scalar2=None,
                                        op0=mybir.AluOpType.mult)
                nc.vector.tensor_scalar(whs[:], he[:], IMG / 2, scalar2=None,
                                        op0=mybir.AluOpType.mult)
                nc.vector.tensor_tensor(rows6[:, hs, 3], cxs[:], whs[:],
                                        op=mybir.AluOpType.subtract)
                nc.vector.tensor_tensor(rows6[:, hs, 5], cxs[:], whs[:],
                                        op=mybir.AluOpType.add)

            # ---- exact rank with +-2 tie window on [8, 260] layout ----
            W = 2
            Vs = pool.tile([8, 256 + 2 * W], mybir.dt.float32)
            Ms = pool.tile([8, 256 + 2 * W], mybir.dt.float32)
            nc.vector.memset(Vs[:], -1.0)
            nc.vector.memset(Ms[:], 0.0)
            # relayout [128,16] -> [8,256] via DRAM staging (partition change)
            vmd = dpool.tile([2048, 2], mybir.dt.float32)
            nc.sync.dma_start(
                vmd[:, 0:1].rearrange("(p c) o -> p (c o)", p=128), conf[:])
            nc.sync.dma_start(
                vmd[:, 1:2].rearrange("(p c) o -> p (c o)", p=128), m_[:])
            nc.sync.dma_start(
                Vs[0:8, W:W + 256],
                vmd[:, 0:1].rearrange("(t q) o -> t (q o)", t=8))
            nc.sync.dma_start(
                Ms[0:8, W:W + 256],
                vmd[:, 1:2].rearrange("(t q) o -> t (q o)", t=8))

            Vc = Vs[:, W:W + 256]
            Mc = Ms[:, W:W + 256]
            rnk = pool.tile([8, 256], mybir.dt.float32)
            ri = pool.tile([8, 256], mybir.dt.int32)
            nc.gpsimd.iota(ri[:], pattern=[[-1, 256]], base=255,
                           channel_multiplier=0)
            nc.vector.tensor_copy(rnk[:], ri[:])  # 255 - q
            eq = pool.tile([8, 256], mybir.dt.float32)
            lt = pool.tile([8, 256], mybir.dt.float32)
            for d in (1, 2, -1, -2):
                Vd = Vs[:, W + d:W + d + 256]
                Md = Ms[:, W + d:W + d + 256]
                nc.vector.tensor_tensor(eq[:], Vc, Vd, op=mybir.AluOpType.is_equal)
                if d > 0:
                    # u term: subtract equal-above count
                    nc.vector.tensor_tensor(rnk[:], rnk[:], eq[:],
                                            op=mybir.AluOpType.subtract)
                nc.vector.tensor_tensor(lt[:], Md, Mc, op=mybir.AluOpType.is_lt)
                nc.vector.tensor_tensor(lt[:], lt[:], eq[:],
                                        op=mybir.AluOpType.mult)
                nc.vector.tensor_tensor(rnk[:], rnk[:], lt[:],
                                        op=mybir.AluOpType.add)

            # route rank back to [128, 16] winner layout via DRAM
            rnkd = dpool.tile([2048, 1], mybir.dt.float32)
            nc.sync.dma_start(
                rnkd[:].rearrange("(t q) o -> t (q o)", t=8), rnk[:])
            rnk128 = pool.tile([128, 16], mybir.dt.float32)
            nc.sync.dma_start(
                rnk128[:], rnkd[:].rearrange("(p c) o -> p (c o)", p=128))

            # dest = t*200 + rank (drop rank >= 200 via bounds_check)
            dest = pool.tile([128, 16], mybir.dt.float32)
            t200 = pool.tile([128, 1], mybir.dt.float32)
            nc.vector.tensor_scalar(t200[:], tf[:], 200.0, scalar2=None,
                                    op0=mybir.AluOpType.mult)
            nc.vector.tensor_scalar(dest[:], rnk128[:], t200[:, 0:1],
                                    scalar2=None, op0=mybir.AluOpType.add)
            big = pool.tile([128, 16], mybir.dt.float32)
            nc.vector.tensor_scalar(big[:], rnk128[:], 199.5, scalar2=1e6,
                                    op0=mybir.AluOpType.is_gt,
                                    op1=mybir.AluOpType.mult)
            nc.vector.tensor_tensor(dest[:], dest[:], big[:],
                                    op=mybir.AluOpType.add)
            destu = pool.tile([128, 16], mybir.dt.uint32)
            nc.vector.tensor_copy(destu[:], dest[:])

            with nc.named_scope("scatter_rows"):
                for k in range(16):
                    nc.gpsimd.indirect_dma_start(
                        out=out[:],
                        out_offset=bass.IndirectOffsetOnAxis(
                            ap=destu[:, k:k + 1], axis=0),
                        in_=rows6[:, k, :], in_offset=None,
                        bounds_check=TPC * TOPK - 1, oob_is_err=False)

    nc.finalize()
    return nc


_NC = None


def kernel(y_pred: np.ndarray, _trace: bool = False) -> np.ndarray:
    global _NC
    y_pred = np.asarray(y_pred, dtype=np.float32)
    assert y_pred.shape == (B, NBOX, NCH)
    if _NC is None:
        _NC = build_kernel()
    in_maps = []
    for c in range(NCORES):
        sl = y_pred[c * TPC:(c + 1) * TPC]          # [8, 8732, 33]
        ypad = np.zeros((TPC, NBP, NCH), np.float32)
        ypad[:, :NBOX] = sl
        in_maps.append({"y": ypad.reshape(TPC * NBP, NCH)})
    res = run_bass_kernel_spmd(_NC, in_maps, core_ids=list(range(NCORES)),
                               trace=_trace)
    kernel._last_results = res
    outs = [r["out"].reshape(TPC, TOPK, 6) for r in res.results]
    return np.concatenate(outs, axis=0)



# revision 3
# speedup vs baseline: 1.5398x; 1.5398x over previous
"""DecodeDetections kernel for Trainium2 (Bass/Tile), 8-core data-parallel.

Full input y_pred [64, 8732, 33] f32 -> output [64, 200, 6] f32.
Each of the 8 NeuronCores handles 8 batch items ("tokens").

Per-core pipeline (partition p = 16t + i holds boxes [546i, 546(i+1)) of
token t):
  1. Strided DMA extracts score channels 1..21 per box directly from DRAM
     into S21 [128, 546*21] (ch 21 is a fake "score" fixed up later).
  2. DVE tensor_reduce(max) over in-box triples -> block maxes BM
     [128, 3824] (3822 real blocks of 3 = 7 per box; pair-block j=6 fixed
     with a 2-wide reduce that excludes the fake channel).
  3. gpsimd topk (vocab 61184) -> exact top-256 blocks per token.
  4. Indirect-gather each winning block's 3 contiguous y elements into
     C [128, 3640]; mask the fake lane of j=6 blocks.
  5. gpsimd topk (vocab 58240) over C -> exact top-256 score values.
  6. Indirect-gather winner block ids, derive (cls, box); gather winner
     rows; decode SSD boxes (Taylor exp).
  7. Exact rank via +-2 tie window on [8, 260] (value desc, m asc);
     blend rows by rank delta and write out[t*200+j] with a reversed-
     stride DMA (no indirect scatter).
"""

import os
import sys

for _p in ("/opt/trn_rl_repo", "/root/.axon_site/_ro/trn_rl_repo"):
    if os.path.isdir(_p) and _p not in sys.path:
        sys.path.insert(0, _p)

import numpy as np

import concourse.bass as bass
import concourse.bacc as bacc
import concourse.bass_isa as bass_isa
import concourse.mybir as mybir
import concourse.tile as tile
from concourse.bass_types import AP
from concourse.bass_utils import run_bass_kernel_spmd

# problem constants
B = 64
NBOX = 8732
NCH = 33
TOPK = 200
NCORES = 8
TPC = 8            # tokens (batch items) per core
NBB = 546          # boxes per partition
NBP = 8736         # padded boxes per token in DRAM
NELEM = TPC * NBP * NCH
SW = NBB * 21      # 11466 score+fake cols per partition
NBLK = NBB * 7     # 3822 blocks per partition
BMW = 3824         # padded block cols (vocab 61184)
CW = 3640          # candidate tile cols (vocab 58240)
IMG = 512.0

f32 = mybir.dt.float32
u32 = mybir.dt.uint32
i32 = mybir.dt.int32


def _topk(nc, out_ap, in_ap, tokens, vocab, k=256):
    _in = nc.gpsimd.lower_ap(in_ap, for_isa=True)
    _out = nc.gpsimd.lower_ap(out_ap, for_isa=True)
    return nc.gpsimd.add_instruction(
        bass_isa.InstTopk(name=f"I-{nc.next_id()}", ins=[_in], outs=[_out],
                          _tokens=tokens, _n=vocab, _k=k))


class _Helper:
    """Float-exact integer div/mod on [128, W] f32 tiles."""

    def __init__(self, nc, pool, w):
        self.nc, self.pool, self.w = nc, pool, w
        self.t1 = pool.tile([128, w], f32, name="hlp_t1")
        self.ti = pool.tile([128, w], i32, name="hlp_ti")
        self.t2 = pool.tile([128, w], f32, name="hlp_t2")

    def fdiv(self, out, in_, d):
        nc = self.nc
        nc.vector.tensor_scalar(self.t1[:], in_, float((1 + 2.0 ** -20) / d),
                                scalar2=None, op0=mybir.AluOpType.mult)
        nc.vector.tensor_copy(self.ti[:], self.t1[:])
        nc.vector.tensor_copy(out, self.ti[:])
        nc.vector.tensor_scalar(self.t1[:], out, float(d),
                                scalar2=None, op0=mybir.AluOpType.mult)
        nc.vector.tensor_tensor(self.t2[:], self.t1[:], in_,
                                op=mybir.AluOpType.is_gt)
        nc.vector.tensor_tensor(out, out, self.t2[:],
                                op=mybir.AluOpType.subtract)

    def fmod(self, out, in_, quot, d):
        nc = self.nc
        nc.vector.tensor_scalar(self.t1[:], quot, float(d),
                                scalar2=None, op0=mybir.AluOpType.mult)
        nc.vector.tensor_tensor(out, in_, self.t1[:],
                                op=mybir.AluOpType.subtract)


def build_kernel():
    nc = bacc.Bacc("TRN2", target_bir_lowering=False, debug=False)
    y = nc.dram_tensor("y", [TPC * NBP, NCH], f32, kind="ExternalInput")
    out = nc.dram_tensor("out", [TPC * TOPK, 6], f32, kind="ExternalOutput")

    TS = mybir.AluOpType
    with tile.TileContext(nc) as tc:
        with tc.tile_pool(name="sbuf", bufs=1) as pool, \
             tc.tile_pool(name="dram", bufs=1, space="DRAM") as dpool:

            # ---- per-partition constants (run while DMAs stream in) ----
            pidx = pool.tile([128, 1], i32)
            nc.gpsimd.iota(pidx[:], pattern=[[0, 1]], base=0,
                           channel_multiplier=1)
            pf = pool.tile([128, 1], f32)
            nc.vector.tensor_copy(pf[:], pidx[:])
            h1 = _Helper(nc, pool, 1)
            tf = pool.tile([128, 1], f32)
            h1.fdiv(tf[:], pf[:], 16)
            t16 = pool.tile([128, 1], f32)
            nc.vector.tensor_scalar(t16[:], tf[:], 16.0, scalar2=None,
                                    op0=TS.mult)
            t256 = pool.tile([128, 1], f32)
            nc.vector.tensor_scalar(t256[:], tf[:], 256.0, scalar2=None,
                                    op0=TS.mult)
            t8736 = pool.tile([128, 1], f32)
            nc.vector.tensor_scalar(t8736[:], tf[:], float(NBP), scalar2=None,
                                    op0=TS.mult)

            BMt = pool.tile([128, BMW], f32)
            nc.vector.memset(BMt[:, NBLK:BMW], 0.0)
            C = pool.tile([128, CW], f32)
            nc.vector.memset(C[:], 0.0)

            # ---- extraction + per-chunk block max ----
            yv = y[:].rearrange("(p b) c -> p b c", p=128)
            bchunks = (0, 137, 274, 410, 546)
            Sc = []
            for j in range(4):
                b0, b1 = bchunks[j], bchunks[j + 1]
                St = pool.tile([128, (b1 - b0) * 21], f32, name=f"S{j}")
                Sc.append((St, b0, b1))
                eng = nc.sync if j % 2 == 0 else nc.scalar
                with nc.named_scope(f"sload{j}"):
                    eng.dma_start(
                        St[:].rearrange("p (b c) -> p b c", c=21),
                        yv[:, b0:b1, 1:22])
            for j, (St, b0, b1) in enumerate(Sc):
                nb = b1 - b0
                with nc.named_scope(f"bmax{j}"):
                    nc.vector.tensor_reduce(
                        BMt[:, b0 * 7:b1 * 7],
                        St[:].rearrange("p (x three) -> p x three", three=3),
                        axis=mybir.AxisListType.X, op=TS.max)
                    nc.vector.tensor_reduce(
                        BMt[:, b0 * 7:b1 * 7].rearrange(
                            "p (b seven) -> p b seven", seven=7)[:, :, 6],
                        St[:].rearrange("p (b c) -> p b c", c=21)[:, :, 18:20],
                        axis=mybir.AxisListType.X, op=TS.max)

            tk1 = pool.tile([128, 32], u32)
            with nc.named_scope("tk1"):
                _topk(nc, tk1[:], BMt[:], tokens=TPC, vocab=BMW * 16)

            # stage block ids to DRAM for the post-merge lookup
            Rd = dpool.tile([128 * 16, 1], u32)
            nc.sync.dma_start(
                Rd[:].rearrange("(p c) o -> p (c o)", p=128), tk1[:, 16:32])

            # ---- winner-block math -> y offsets of the 3 elements ----
            h16 = _Helper(nc, pool, 16)
            r1f = pool.tile([128, 16], f32)
            nc.vector.tensor_copy(r1f[:], tk1[:, 16:32])
            i1 = pool.tile([128, 16], f32)
            x1 = pool.tile([128, 16], f32)
            b1t = pool.tile([128, 16], f32)
            j1 = pool.tile([128, 16], f32)
            h16.fdiv(i1[:], r1f[:], BMW)
            h16.fmod(x1[:], r1f[:], i1[:], BMW)
            h16.fdiv(b1t[:], x1[:], 7)
            h16.fmod(j1[:], x1[:], b1t[:], 7)
            u = pool.tile([128, 16], f32)
            o1 = pool.tile([128, 16], f32)
            nc.vector.tensor_scalar(u[:], i1[:], t16[:, 0:1], scalar2=None,
                                    op0=TS.add)
            nc.vector.tensor_scalar(u[:], u[:], float(NBB), scalar2=None,
                                    op0=TS.mult)
            nc.vector.tensor_tensor(u[:], u[:], b1t[:], op=TS.add)
            nc.vector.tensor_scalar(u[:], u[:], 33.0, scalar2=None,
                                    op0=TS.mult)
            nc.vector.tensor_scalar(o1[:], j1[:], 3.0, scalar2=1.0,
                                    op0=TS.mult, op1=TS.add)
            nc.vector.tensor_tensor(o1[:], o1[:], u[:], op=TS.add)
            offsu = pool.tile([128, 16], u32)
            nc.vector.tensor_copy(offsu[:], o1[:])

            # ---- gather candidate triples ----
            ysrc = AP(tensor=y[:].tensor, offset=0,
                      ap=[[1, NELEM], [1, 3]])
            with nc.named_scope("candgather"):
                for k in range(16):
                    nc.gpsimd.indirect_dma_start(
                        out=C[:, 3 * k:3 * k + 3], out_offset=None, in_=ysrc,
                        in_offset=bass.IndirectOffsetOnAxis(
                            ap=offsu[:, k:k + 1], axis=1),
                        bounds_check=NELEM - 3, oob_is_err=False)
            # fake-lane mask for pair blocks (j == 6)
            isj6 = pool.tile([128, 16], f32)
            nc.vector.tensor_scalar(isj6[:], j1[:], 5.5, scalar2=-1.0,
                                    op0=TS.is_gt, op1=TS.mult)
            nc.vector.tensor_scalar(isj6[:], isj6[:], 1.0, scalar2=None,
                                    op0=TS.add)
            cview = C[:, 0:48].rearrange("p (k three) -> p k three",
                                         three=3)[:, :, 2]
            nc.vector.tensor_tensor(cview, cview, isj6[:], op=TS.mult)

            tk2 = pool.tile([128, 32], u32)
            with nc.named_scope("tk2"):
                _topk(nc, tk2[:], C[:], tokens=TPC, vocab=CW * 16)

            # ---- winner math: position in C -> block slot -> block id ----
            v2f = pool.tile([128, 16], f32)
            nc.vector.tensor_copy(v2f[:], tk2[:, 16:32])
            i2 = pool.tile([128, 16], f32)
            c2 = pool.tile([128, 16], f32)
            k2 = pool.tile([128, 16], f32)
            w2 = pool.tile([128, 16], f32)
            h16.fdiv(i2[:], v2f[:], CW)
            h16.fmod(c2[:], v2f[:], i2[:], CW)
            h16.fdiv(k2[:], c2[:], 3)
            h16.fmod(w2[:], c2[:], k2[:], 3)
            Ff = pool.tile([128, 16], f32)
            nc.vector.tensor_scalar(Ff[:], i2[:], 16.0, scalar2=None,
                                    op0=TS.mult)
            nc.vector.tensor_tensor(Ff[:], Ff[:], k2[:], op=TS.add)
            nc.vector.tensor_scalar(Ff[:], Ff[:], t256[:, 0:1], scalar2=None,
                                    op0=TS.add)
            Fu = pool.tile([128, 16], u32)
            nc.vector.tensor_copy(Fu[:], Ff[:])

            rhw = pool.tile([128, 16], u32)
            with nc.named_scope("vgather"):
                for k in range(16):
                    nc.gpsimd.indirect_dma_start(
                        out=rhw[:, k:k + 1], out_offset=None, in_=Rd[:],
                        in_offset=bass.IndirectOffsetOnAxis(
                            ap=Fu[:, k:k + 1], axis=0),
                        bounds_check=128 * 16 - 1, oob_is_err=False)

            # ---- decode winners: block id + w -> (cls, n, m, yrow) ----
            rwf = pool.tile([128, 16], f32)
            nc.vector.tensor_copy(rwf[:], rhw[:])
            iw = pool.tile([128, 16], f32)
            xw = pool.tile([128, 16], f32)
            bw = pool.tile([128, 16], f32)
            jw = pool.tile([128, 16], f32)
            h16.fdiv(iw[:], rwf[:], BMW)
            h16.fmod(xw[:], rwf[:], iw[:], BMW)
            h16.fdiv(bw[:], xw[:], 7)
            h16.fmod(jw[:], xw[:], bw[:], 7)
            cl = pool.tile([128, 16], f32)
            nc.vector.tensor_scalar(cl[:], jw[:], 3.0, scalar2=1.0,
                                    op0=TS.mult, op1=TS.add)
            nc.vector.tensor_tensor(cl[:], cl[:], w2[:], op=TS.add)
            nf = pool.tile([128, 16], f32)
            nc.vector.tensor_scalar(nf[:], iw[:], float(NBB), scalar2=None,
                                    op0=TS.mult)
            nc.vector.tensor_tensor(nf[:], nf[:], bw[:], op=TS.add)
            m_ = pool.tile([128, 16], f32)
            nc.vector.tensor_scalar(m_[:], cl[:], float(NBOX), scalar2=None,
                                    op0=TS.mult)
            nc.vector.tensor_tensor(m_[:], m_[:], nf[:], op=TS.add)
            yrow = pool.tile([128, 16], f32)
            nc.vector.tensor_scalar(yrow[:], nf[:], t8736[:, 0:1],
                                    scalar2=None, op0=TS.add)
            rowu = pool.tile([128, 16], u32)
            nc.vector.tensor_copy(rowu[:], yrow[:])
            conf = pool.tile([128, 16], f32)
            nc.vector.tensor_copy(conf[:], tk2[:, 0:16].bitcast(f32))

            # ---- rank path staging (sync queue; overlaps row gather) ----
            vmd = dpool.tile([2048, 2], f32)
            nc.sync.dma_start(
                vmd[:, 0:1].rearrange("(p c) o -> p (c o)", p=128), conf[:])
            nc.sync.dma_start(
                vmd[:, 1:2].rearrange("(p c) o -> p (c o)", p=128), m_[:])
            W = 2
            Vs = pool.tile([8, 256 + 2 * W], f32)
            Ms = pool.tile([8, 256 + 2 * W], f32)
            nc.vector.memset(Vs[:], -1.0)
            nc.vector.memset(Ms[:], 0.0)
            nc.sync.dma_start(
                Vs[0:8, W:W + 256],
                vmd[:, 0:1].rearrange("(t q) o -> t (q o)", t=8))
            nc.sync.dma_start(
                Ms[0:8, W:W + 256],
                vmd[:, 1:2].rearrange("(t q) o -> t (q o)", t=8))

            # ---- row gather ----
            enc = pool.tile([128, 16, NCH], f32)
            with nc.named_scope("rowgather"):
                for k in range(16):
                    nc.gpsimd.indirect_dma_start(
                        out=enc[:, k, :], out_offset=None, in_=y[:],
                        in_offset=bass.IndirectOffsetOnAxis(
                            ap=rowu[:, k:k + 1], axis=0),
                        bounds_check=TPC * NBP - 1, oob_is_err=False)

            # ---- rank delta DL on [8, 256] (runs on DVE during gather) ----
            Vc = Vs[:, W:W + 256]
            Mc = Ms[:, W:W + 256]
            DL = pool.tile([8, 256], f32)
            nc.vector.memset(DL[:], 0.0)
            eq = pool.tile([8, 256], f32)
            lt = pool.tile([8, 256], f32)
            for d in (1, 2, -1, -2):
                Vd = Vs[:, W + d:W + d + 256]
                Md = Ms[:, W + d:W + d + 256]
                nc.vector.tensor_tensor(eq[:], Vc, Vd, op=TS.is_equal)
                if d > 0:
                    nc.vector.tensor_tensor(DL[:], DL[:], eq[:],
                                            op=TS.subtract)
                nc.vector.tensor_tensor(lt[:], Md, Mc, op=TS.is_lt)
                nc.vector.tensor_tensor(lt[:], lt[:], eq[:], op=TS.mult)
                nc.vector.tensor_tensor(DL[:], DL[:], lt[:], op=TS.add)
            DLp = pool.tile([8, 256 + 2 * W], f32)
            nc.vector.memset(DLp[:], 99.0)
            nc.vector.tensor_copy(DLp[:, W:W + 256], DL[:])

            # ---- box decode (Taylor exp, identical to reference math) ----
            import math as _math
            EXP_C = [1.0 / _math.factorial(kk) for kk in range(11)]

            def ch(k):
                return enc[:, :, 21 + k]

            cx = pool.tile([128, 16], f32)
            cy = pool.tile([128, 16], f32)
            we = pool.tile([128, 16], f32)
            he = pool.tile([128, 16], f32)
            nc.vector.tensor_tensor(cx[:], ch(0), ch(8), op=TS.mult)
            nc.vector.tensor_tensor(cx[:], cx[:], ch(6), op=TS.mult)
            nc.vector.tensor_tensor(cx[:], cx[:], ch(4), op=TS.add)
            nc.vector.tensor_tensor(cy[:], ch(1), ch(9), op=TS.mult)
            nc.vector.tensor_tensor(cy[:], cy[:], ch(7), op=TS.mult)
            nc.vector.tensor_tensor(cy[:], cy[:], ch(5), op=TS.add)
            nc.vector.tensor_tensor(we[:], ch(2), ch(10), op=TS.mult)
            nc.vector.tensor_tensor(he[:], ch(3), ch(11), op=TS.mult)
            xe = pool.tile([128, 32], f32)
            nc.vector.tensor_copy(xe[:, 0:16], we[:])
            nc.vector.tensor_copy(xe[:, 16:32], he[:])
            acc = pool.tile([128, 32], f32)
            nc.vector.memset(acc[:], EXP_C[10])
            for kk in range(9, -1, -1):
                nc.vector.tensor_tensor(acc[:], acc[:], xe[:], op=TS.mult)
                nc.vector.tensor_scalar(acc[:], acc[:], EXP_C[kk],
                                        scalar2=None, op0=TS.add)
            nc.vector.tensor_tensor(we[:], acc[:, 0:16], ch(6), op=TS.mult)
            nc.vector.tensor_tensor(he[:], acc[:, 16:32], ch(7), op=TS.mult)

            R6 = pool.tile([128, 16, 6], f32)
            nc.vector.tensor_copy(R6[:, :, 0], cl[:])
            nc.vector.tensor_copy(R6[:, :, 1], conf[:])
            cxs = pool.tile([128, 16], f32)
            whs = pool.tile([128, 16], f32)
            nc.vector.tensor_scalar(cxs[:], cx[:], IMG, scalar2=None,
                                    op0=TS.mult)
            nc.vector.tensor_scalar(whs[:], we[:], IMG / 2, scalar2=None,
                                    op0=TS.mult)
            nc.vector.tensor_tensor(R6[:, :, 2], cxs[:], whs[:],
                                    op=TS.subtract)
            nc.vector.tensor_tensor(R6[:, :, 4], cxs[:], whs[:], op=TS.add)
            nc.vector.tensor_scalar(cxs[:], cy[:], IMG, scalar2=None,
                                    op0=TS.mult)
            nc.vector.tensor_scalar(whs[:], he[:], IMG / 2, scalar2=None,
                                    op0=TS.mult)
            nc.vector.tensor_tensor(R6[:, :, 3], cxs[:], whs[:],
                                    op=TS.subtract)
            nc.vector.tensor_tensor(R6[:, :, 5], cxs[:], whs[:], op=TS.add)

            # ---- blend rows by rank delta, write reversed ----
            rows6d = dpool.tile([2048, 6], f32)
            nc.sync.dma_start(
                rows6d[:].rearrange("(p k) c -> p (k c)", p=128),
                R6[:].rearrange("p k c -> p (k c)"))
            Rp = pool.tile([8, 260 * 6], f32)
            nc.vector.memset(Rp[:, 0:12], 0.0)
            nc.vector.memset(Rp[:, 1548:1560], 0.0)
            nc.sync.dma_start(
                Rp[:, 12:1548].rearrange("t (q c) -> t q c", c=6),
                rows6d[:].rearrange("(t q) c -> t q c", t=8))
            G = pool.tile([8, 256 * 6], f32)
            nc.vector.memset(G[:], 0.0)
            cmp = pool.tile([8, 256], f32)
            cmpi = pool.tile([8, 256], i32)
            for e in (-2, -1, 0, 1, 2):
                nc.vector.tensor_scalar(cmp[:], DLp[:, W + e:W + e + 256],
                                        float(e), scalar2=None,
                                        op0=TS.is_equal)
                nc.vector.tensor_copy(cmpi[:], cmp[:])
                nc.vector.copy_predicated(
                    G[:].rearrange("t (q c) -> t q c", c=6),
                    cmpi[:].rearrange("t (q o) -> t q o", o=1).to_broadcast(
                        [8, 256, 6]),
                    Rp[:, (W + e) * 6:(W + e + 256) * 6].rearrange(
                        "t (q c) -> t q c", c=6))

            gsrc = AP(tensor=G.tensor, offset=255 * 6,
                      ap=[[256 * 6, 8], [-6, 200], [1, 6]])
            with nc.named_scope("outw"):
                nc.sync.dma_start(
                    out[:].rearrange("(t j) c -> t j c", t=8), gsrc)

    nc.finalize()
    return nc


_NC = None


def kernel(y_pred: np.ndarray, _trace: bool = False) -> np.ndarray:
    global _NC
    y_pred = np.asarray(y_pred, dtype=np.float32)
    assert y_pred.shape == (B, NBOX, NCH)
    if _NC is None:
        _NC = build_kernel()
    in_maps = []
    for c in range(NCORES):
        sl = y_pred[c * TPC:(c + 1) * TPC]          # [8, 8732, 33]
        ypad = np.zeros((TPC, NBP, NCH), np.float32)
        ypad[:, :NBOX] = sl
        in_maps.append({"y": ypad.reshape(TPC * NBP, NCH)})
    res = run_bass_kernel_spmd(_NC, in_maps, core_ids=list(range(NCORES)),
                               trace=_trace)
    kernel._last_results = res
    outs = [r["out"].reshape(TPC, TOPK, 6) for r in res.results]
    return np.concatenate(outs, axis=0)
